# revision 1
# baseline (speedup 1.0000x reference)
"""MultiHeadAttention kernel for 8x TRN2 NeuronCores.

The reference module's einsum reduces the attention tensor over BOTH the
query and key axes (attn_mass = sum_{q,k} softmax(logits)_k), and softmax
rows sum to 1, so attn_mass == Lq exactly for every (batch, head). The
whole computation therefore collapses to

    out = (Lq * (V_heads @ Wv^T + bv)).reshape(N, L, E) @ Wo^T + bo

which is a single dense GEMM after folding the (block-diagonal) per-head
V-projection into the output projection:

    out = V_flat @ W_eff + b_eff
    W_eff[h*hd+a, n] = Lq * sum_b Wv[b, a] * Wo[n, h*hd+b]      (1024 x 1024)
    b_eff[n]         = Lq * sum_{h,b} Wo[n, h*hd+b] * bv[b] + bo[n]

The device kernel is the GEMM, row-sharded across 8 cores (512 rows per
core), computed in TRANSPOSED orientation: out^T[n, m] = sum_k W[k, n]
X[m, k].  Each PSUM bank j holds output columns j*128..(j+1)*128 on
partitions x all 512 rows on the free dim, accumulating lhsT = W-block j
(natural layout) against rhs = X^T k-slabs.  Benefits:

  * bias varies along PARTITIONS -> folded into the PSUM eviction as a
    free per-partition tensor_scalar_add on the vector engine;
  * input stream order [bias+warm | W0+X0 head | X 1-7 | W-blocks 1-7]
    lets bank j finish as soon as W-block j lands, so output DMAs
    overlap the input stream (bulk input DMAs drain through the sync
    engine's FIFO HWDGE queue at ~350 GB/s; the head rides the scalar
    engine's own HWDGE queue to unblock bank 0 early);
  * a few K=128 junk matmuls on real (nonzero!) fp32 data warm the PE
    HAM clock gate before the first real matmul (zero data is
    activity-gated and does not warm the clock; K=1 matmuls do not
    register either).

The host transposes V-shards in, and the (E, RPC) per-core outputs back.
"""

import numpy as np

import concourse.bass as bass
import concourse.bacc as bacc
import concourse.mybir as mybir
from concourse.tile import TileContext
from concourse.bass_utils import run_bass_kernel_spmd

N_CORES = 8
E = 1024            # embed dim == d_model
H, HD = 16, 64      # heads, head dim
ROWS = 4096         # N * L = 2 * 2048
RPC = ROWS // N_CORES   # rows per core = 512
P = 128             # SBUF partitions
KT = E // P         # 8 contraction slabs
JT = E // P         # 8 output-column banks
N_WARM = 11         # K=128 fp32 junk matmuls for PE HAM warm-up

_NC_CACHE = {}
LAST_RESULTS = None  # BassKernelResults of the most recent device run


def _build(dtype):
    f32 = mybir.dt.float32
    nc = bacc.Bacc(None, target_bir_lowering=False)
    # head packs [W-block0 | X-slab0] so one DMA (on the scalar engine's
    # own HWDGE queue, concurrent with the bulk stream) unblocks bank 0.
    head = nc.declare_dram_parameter("head", [P, E + RPC], dtype, isOutput=False)
    xs = nc.declare_dram_parameter("xs", [E, RPC], dtype, isOutput=False)
    wc = nc.declare_dram_parameter("wc", [JT * P, E], dtype, isOutput=False)
    # bw packs bias columns (JT) and a warm-up block (P) side by side.
    bw = nc.declare_dram_parameter("bw", [P, JT + P], f32, isOutput=False)
    outT = nc.declare_dram_parameter("outT", [E, RPC], f32, isOutput=True)

    with TileContext(nc) as tc:
        with (
            tc.tile_pool(name="xp", bufs=1) as xp,
            tc.tile_pool(name="wp", bufs=1) as wp,
            tc.tile_pool(name="bp", bufs=1) as bp,
            tc.tile_pool(name="pp", bufs=1, space="PSUM") as pp,
            tc.tile_pool(name="op", bufs=1) as op,
        ):
            # memset needs no DMA: junk matmuls can start right after the
            # BSP preamble, well before any input data lands.
            wm_t = bp.tile([P, P], f32, name="wm", tag="wm")
            nc.gpsimd.memset(wm_t[:], 1.0)
            bw_t = bp.tile([P, JT + P], f32, name="bw", tag="bw")

            # head [W0|X0] rides the scalar engine's HWDGE queue; the
            # sync queue interleaves W-blocks into the X stream so the
            # PE (fed in data-arrival order below) never starves, with
            # W7 last (only bank 7 trails the stream).  xrank/wrank
            # mirror the FIFO arrival order of each operand.
            # X-priority dual-queue: all of X lands first across BOTH
            # HWDGE queues (head+x1-3 on the scalar queue, x4-7 leading
            # the sync queue), so every bank's k7 unlocks early and the
            # banks then pace off their W-block arrivals, nicely spread.
            head_t = bp.tile([P, E + RPC], dtype, name="head", tag="head")
            nc.scalar.dma_start(out=head_t[:], in_=head[:, :])
            wts = [None] * JT
            wts[0] = head_t[:, 0:E]
            xts = [head_t[:, E:E + RPC]]
            for k in range(1, KT):
                t = xp.tile([P, RPC], dtype, name=f"x{k}", tag=f"x{k}")
                xts.append(t)
            # All of X rides the fast sync queue (bank 0 unblocks ~16us);
            # w1 and w3 ride the slower scalar queue behind the head and
            # land just before banks 1/3 need them; the remaining W
            # blocks follow X on the sync queue with ~2us of margin each.
            for k in range(1, KT):
                nc.sync.dma_start(out=xts[k][:], in_=xs[k * P:(k + 1) * P, :])
            for j in (1, 3):
                wts[j] = wp.tile([P, E], dtype, name=f"w{j}", tag=f"w{j}")
                nc.scalar.dma_start(out=wts[j][:], in_=wc[j * P:(j + 1) * P, :])
            # bias+warm block rides the scalar queue too: off the sync
            # queue's critical X phase, still ~2us ahead of first eviction
            nc.scalar.dma_start(out=bw_t[:], in_=bw[:, :])
            for j in (2, 4, 5, 6):
                wts[j] = wp.tile([P, E], dtype, name=f"w{j}", tag=f"w{j}")
                nc.sync.dma_start(out=wts[j][:], in_=wc[j * P:(j + 1) * P, :])
            # last W block as four separate quarter tiles so bank 7's
            # matmuls chase the quarters as they land
            q = E // 4
            w7q = []
            for c in range(4):
                t = wp.tile([P, q], dtype, name=f"w7q{c}", tag=f"w7q{c}")
                nc.sync.dma_start(
                    out=t[:], in_=wc[(JT - 1) * P:JT * P, c * q:(c + 1) * q]
                )
                w7q.append(t)

            ps = [
                pp.tile([P, RPC], f32, name=f"ps{j}", tag=f"ps{j}")
                for j in range(JT)
            ]

            # PE warm-up on nonzero fp32 data (4 cycles/row -- dense HAM
            # activity) starting right after the preamble, so the HAM
            # clock-gate lifts before the first real matmul.
            for i in range(N_WARM):
                nc.tensor.matmul(
                    ps[i % JT][:, 0:P],
                    wm_t[:, :],
                    wm_t[:, :],
                    start=True,
                    stop=True,
                )

            # Bank-major emission: bank j is gated by its own W block
            # (X has fully landed by then), so banks finish ~evenly
            # spread and their output DMAs overlap the tail.
            def lhsT(j, k):
                if j < JT - 1:
                    return wts[j][:, k * P:(k + 1) * P]
                c = k // 2
                return w7q[c][:, (k - 2 * c) * P:(k - 2 * c + 1) * P]

            for j in range(JT):
                for k in range(KT):
                    nc.tensor.matmul(
                        ps[j],
                        lhsT(j, k),
                        xts[k][:, :],
                        start=(k == 0),
                        stop=(k == KT - 1),
                    )
            for j in range(JT):
                o = op.tile([P, RPC], f32, name=f"o{j}", tag=f"o{j}")
                if j < JT - 1:
                    nc.vector.tensor_scalar_add(o[:], ps[j], bw_t[:, j:j + 1])
                    # HWDGE FIFO: enqueues behind any remaining input
                    # DMAs; only the LAST bank's output is a deadline,
                    # and it issues after the input stream has drained.
                    nc.sync.dma_start(out=outT[j * P:(j + 1) * P, :], in_=o[:])
                else:
                    # halve the final eviction so its first output DMA
                    # overlaps the second half's tensor_scalar_add
                    hh = RPC // 2
                    for c in range(2):
                        nc.vector.tensor_scalar_add(
                            o[:, c * hh:(c + 1) * hh],
                            ps[j][:, c * hh:(c + 1) * hh],
                            bw_t[:, j:j + 1],
                        )
                        nc.sync.dma_start(
                            out=outT[j * P:(j + 1) * P, c * hh:(c + 1) * hh],
                            in_=o[:, c * hh:(c + 1) * hh],
                        )
    nc.compile()
    return nc


def _get_nc(dtype_name):
    if dtype_name not in _NC_CACHE:
        _NC_CACHE[dtype_name] = _build(getattr(mybir.dt, dtype_name))
    return _NC_CACHE[dtype_name]


def _prep_in_maps(V, Wv, bv, Wo, bo, lq):
    V = np.ascontiguousarray(np.asarray(V, dtype=np.float32))
    Wv64 = np.asarray(Wv, np.float64)
    Wo64 = np.asarray(Wo, np.float64)
    bv64 = np.asarray(bv, np.float64)
    bo64 = np.asarray(bo, np.float64)

    # Fold per-head V-projection + output projection + attention mass (== Lq).
    Wo_r = Wo64.reshape(E, H, HD)                       # [n, h, b]
    W_eff = lq * np.einsum("ba,nhb->han", Wv64, Wo_r, optimize=True)
    W_eff = W_eff.reshape(E, E).astype(np.float32)      # [k, n]
    b_eff = (lq * np.einsum("nhb,b->n", Wo_r, bv64) + bo64).astype(np.float32)

    # wc[j*P + p, k*P + c] = W_eff[k*P + p, j*P + c]  (lhsT blocks, natural)
    wc = np.ascontiguousarray(
        W_eff.reshape(KT, P, JT, P).transpose(2, 1, 0, 3).reshape(JT * P, E)
    )
    bw_blk = np.ones((P, JT + P), np.float32)
    bw_blk[:, :JT] = b_eff.reshape(JT, P).T                 # [p, j]

    X = V.reshape(ROWS, E)
    in_maps = []
    for i in range(N_CORES):
        xs_i = np.ascontiguousarray(X[i * RPC:(i + 1) * RPC, :].T)
        head_i = np.empty((P, E + RPC), np.float32)
        head_i[:, :E] = wc[0:P, :]
        head_i[:, E:] = xs_i[0:P, :]
        in_maps.append({"head": head_i, "xs": xs_i, "wc": wc, "bw": bw_blk})
    return in_maps


def kernel(Q, K, V, Wq, bq, Wk, bk, Wv, bv, Wo, bo, dtype_name="float32r", **_unused):
    global LAST_RESULTS
    n, L, e = np.asarray(V).shape
    lq = float(np.asarray(Q).shape[1])
    in_maps = _prep_in_maps(V, Wv, bv, Wo, bo, lq)
    nc = _get_nc(dtype_name)
    LAST_RESULTS = run_bass_kernel_spmd(nc, in_maps, list(range(N_CORES)))
    out = np.concatenate(
        [LAST_RESULTS.results[i]["outT"].T for i in range(N_CORES)], axis=0
    )
    return np.ascontiguousarray(out).reshape(n, L, E)



# revision 2
# speedup vs baseline: 1.1410x; 1.1410x over previous
"""MultiHeadAttention kernel for 8x TRN2 NeuronCores.

The reference module's einsum reduces the attention tensor over BOTH the
query and key axes (attn_mass = sum_{q,k} softmax(logits)_k), and softmax
rows sum to 1, so attn_mass == Lq exactly for every (batch, head). The
whole computation therefore collapses to

    out = (Lq * (V_heads @ Wv^T + bv)).reshape(N, L, E) @ Wo^T + bo

which is a single dense GEMM after folding the (block-diagonal) per-head
V-projection into the output projection:

    out = V_flat @ W_eff + b_eff
    W_eff[h*hd+a, n] = Lq * sum_b Wv[b, a] * Wo[n, h*hd+b]      (1024 x 1024)
    b_eff[n]         = Lq * sum_{h,b} Wo[n, h*hd+b] * bv[b] + bo[n]

The device kernel is the GEMM, row-sharded across 8 cores (512 rows per
core), computed in TRANSPOSED orientation: out^T[n, m] = sum_k W[k, n]
X[m, k].  PSUM bank j holds output columns j*128..(j+1)*128 on partitions
x all 512 rows on the free dim, accumulating lhsT = W-block j against
rhs = X^T k-slabs.

v2 (this file): everything rides bf16 (inputs, weights, output — PSUM
still accumulates fp32; 2e-2 tolerance leaves ~5x margin), halving HBM
traffic, and the schedule is rebuilt around the two real bottlenecks the
fp32 trace exposed:

  * HAM clock ramp: the PE runs at ~1.2 GHz until it has been
    continuously busy ~4us, and a mid-stream DMA stall re-cools it
    (the fp32 run paid ~10us at half clock after stalling).  So: a
    bf16 junk-matmul burst starts the ramp right after the preamble
    and is sized so the first real matmul's inputs have landed by the
    time it drains — the PE never idles once started.
  * DMA supply: inputs stream over THREE queues (sync HWDGE, scalar
    HWDGE, gpsimd SWDGE), each tile ordered by its consumption
    deadline.  Banks 0 and 1 are interleaved (k-offset 2) so the
    X-slab consumption rate during the arrival phase is halved.
  * Output is bf16 too (host upcasts): banks evict through the vector
    engine (bias add fused, fp32->bf16) and drain on the sync queue,
    with the last bank split into quarters to shave the tail.
"""

import numpy as np
import ml_dtypes

import concourse.bass as bass
import concourse.bacc as bacc
import concourse.mybir as mybir
from concourse.tile import TileContext
from concourse.bass_utils import run_bass_kernel_spmd

N_CORES = 8
E = 1024            # embed dim == d_model
H, HD = 16, 64      # heads, head dim
ROWS = 4096         # N * L = 2 * 2048
RPC = ROWS // N_CORES   # rows per core = 512
P = 128             # SBUF partitions
KT = E // P         # 8 contraction slabs
JT = E // P         # 8 output-column banks

# Junk-matmul warm-up burst: keeps the PE busy (HAM ramp) from preamble
# exit until the first real operands land (~3.4us at the mid p-state).
N_JUNK_512 = 8
N_JUNK_128 = 2

# Interleave banks 0 and 1 (bank-1 lags by 2 k-steps) so X slabs are
# consumed every ~2 matmuls during the DMA arrival phase; banks 2..7
# run bank-major once all X is resident.
MM_ORDER = [
    (0, 0), (0, 1), (1, 0), (0, 2), (1, 1), (0, 3), (1, 2), (0, 4),
    (1, 3), (0, 5), (1, 4), (0, 6), (1, 5), (0, 7), (1, 6), (1, 7),
] + [(j, k) for j in range(2, JT) for k in range(KT)]

_NC_CACHE = {}
LAST_RESULTS = None  # BassKernelResults of the most recent device run


def _build():
    f32 = mybir.dt.float32
    bf16 = mybir.dt.bfloat16
    nc = bacc.Bacc(None, target_bir_lowering=False)
    xs = nc.declare_dram_parameter("xs", [E, RPC], bf16, isOutput=False)
    wc = nc.declare_dram_parameter("wc", [JT * P, E], bf16, isOutput=False)
    bw = nc.declare_dram_parameter("bw", [P, JT], f32, isOutput=False)
    outT = nc.declare_dram_parameter("outT", [E, RPC], bf16, isOutput=True)

    with TileContext(nc) as tc:
        with (
            tc.tile_pool(name="xp", bufs=1) as xp,
            tc.tile_pool(name="wp", bufs=1) as wp,
            tc.tile_pool(name="bp", bufs=1) as bp,
            tc.tile_pool(name="pp", bufs=1, space="PSUM") as pp,
            tc.tile_pool(name="op", bufs=1) as op,
        ):
            # Junk tile for the warm-up burst: memset needs no DMA, so the
            # PE can start right after the preamble.  Vector does the
            # memset (gpsimd is busy generating SWDGE descriptors).
            wm = bp.tile([P, RPC], bf16, name="wm", tag="wm")
            nc.vector.memset(wm[:], 1.0)
            bias = bp.tile([P, JT], f32, name="bias", tag="bias")

            xts = [
                xp.tile([P, RPC], bf16, name=f"x{k}", tag=f"x{k}")
                for k in range(KT)
            ]

            # W chunk tiles.  W0/W1 are split into small chunks so their
            # first k-slabs can land (and unblock the PE) early; W2..W7
            # are whole blocks.  wmap[(j, k)] = (tile, col offset).
            wmap = {}

            def wchunk(j, k0, k1, engine):
                t = wp.tile([P, (k1 - k0) * P], bf16, name=f"w{j}_{k0}{k1}",
                            tag=f"w{j}_{k0}{k1}")
                engine.dma_start(
                    out=t[:], in_=wc[j * P:(j + 1) * P, k0 * P:k1 * P]
                )
                for k in range(k0, k1):
                    wmap[(j, k)] = (t, (k - k0) * P)

            # --- DMA schedule: three queues, deadline order -------------
            # gpsimd SWDGE: first-k chunks of W0/W1, then W1 tail + W7.
            wchunk(0, 0, 2, nc.gpsimd)
            wchunk(1, 0, 2, nc.gpsimd)
            wchunk(1, 2, 4, nc.gpsimd)
            wchunk(1, 4, 8, nc.gpsimd)
            wchunk(7, 0, 8, nc.gpsimd)
            # sync HWDGE: even X slabs interleaved with W0 tail, then
            # even W blocks.  Output DMAs are queued here afterwards.
            nc.sync.dma_start(out=xts[0][:], in_=xs[0:P, :])
            wchunk(0, 2, 4, nc.sync)
            nc.sync.dma_start(out=xts[2][:], in_=xs[2 * P:3 * P, :])
            wchunk(0, 4, 6, nc.sync)
            nc.sync.dma_start(out=xts[4][:], in_=xs[4 * P:5 * P, :])
            wchunk(0, 6, 8, nc.sync)
            nc.sync.dma_start(out=xts[6][:], in_=xs[6 * P:7 * P, :])
            wchunk(2, 0, 8, nc.sync)
            wchunk(4, 0, 8, nc.sync)
            wchunk(6, 0, 8, nc.sync)
            # scalar HWDGE: bias (tiny, unblocks evictions), odd X slabs,
            # then odd W blocks.
            nc.scalar.dma_start(out=bias[:], in_=bw[:, :])
            nc.scalar.dma_start(out=xts[1][:], in_=xs[P:2 * P, :])
            nc.scalar.dma_start(out=xts[3][:], in_=xs[3 * P:4 * P, :])
            nc.scalar.dma_start(out=xts[5][:], in_=xs[5 * P:6 * P, :])
            nc.scalar.dma_start(out=xts[7][:], in_=xs[7 * P:8 * P, :])
            wchunk(3, 0, 8, nc.scalar)
            wchunk(5, 0, 8, nc.scalar)

            ps = [
                pp.tile([P, RPC], f32, name=f"ps{j}", tag=f"ps{j}")
                for j in range(JT)
            ]

            # Warm-up burst: nonzero bf16 junk matmuls, no DMA deps.
            for i in range(N_JUNK_512):
                nc.tensor.matmul(
                    ps[i % JT], wm[:, 0:P], wm[:, :], start=True, stop=True
                )
            for i in range(N_JUNK_128):
                nc.tensor.matmul(
                    ps[(N_JUNK_512 + i) % JT][:, 0:P],
                    wm[:, 0:P], wm[:, 0:P], start=True, stop=True,
                )

            for j, k in MM_ORDER:
                t, off = wmap[(j, k)]
                nc.tensor.matmul(
                    ps[j],
                    t[:, off:off + P],
                    xts[k][:, :],
                    start=(k == 0),
                    stop=(k == KT - 1),
                )

            # Evictions: fused bias add fp32->bf16 on vector, out DMAs on
            # the (by now idle) sync queue.  Last bank in quarters so its
            # first chunks stream while the rest are still evicting.
            for j in range(JT):
                o = op.tile([P, RPC], bf16, name=f"o{j}", tag=f"o{j}")
                if j < JT - 1:
                    nc.vector.tensor_scalar_add(o[:], ps[j], bias[:, j:j + 1])
                    nc.sync.dma_start(out=outT[j * P:(j + 1) * P, :], in_=o[:])
                else:
                    q = RPC // 4
                    for c in range(4):
                        nc.vector.tensor_scalar_add(
                            o[:, c * q:(c + 1) * q],
                            ps[j][:, c * q:(c + 1) * q],
                            bias[:, j:j + 1],
                        )
                        nc.sync.dma_start(
                            out=outT[j * P:(j + 1) * P, c * q:(c + 1) * q],
                            in_=o[:, c * q:(c + 1) * q],
                        )
    nc.compile()
    return nc


def _get_nc():
    if "bf16" not in _NC_CACHE:
        _NC_CACHE["bf16"] = _build()
    return _NC_CACHE["bf16"]


def _prep_in_maps(V, Wv, bv, Wo, bo, lq):
    V = np.ascontiguousarray(np.asarray(V, dtype=np.float32))
    Wv64 = np.asarray(Wv, np.float64)
    Wo64 = np.asarray(Wo, np.float64)
    bv64 = np.asarray(bv, np.float64)
    bo64 = np.asarray(bo, np.float64)

    # Fold per-head V-projection + output projection + attention mass (== Lq).
    Wo_r = Wo64.reshape(E, H, HD)                       # [n, h, b]
    W_eff = lq * np.einsum("ba,nhb->han", Wv64, Wo_r, optimize=True)
    W_eff = W_eff.reshape(E, E).astype(np.float32)      # [k, n]
    b_eff = (lq * np.einsum("nhb,b->n", Wo_r, bv64) + bo64).astype(np.float32)

    # wc[j*P + p, k*P + c] = W_eff[k*P + p, j*P + c]  (lhsT blocks, natural)
    wc = np.ascontiguousarray(
        W_eff.reshape(KT, P, JT, P).transpose(2, 1, 0, 3).reshape(JT * P, E)
    ).astype(ml_dtypes.bfloat16)
    bw_blk = np.ascontiguousarray(b_eff.reshape(JT, P).T)   # [p, j] fp32

    X = V.reshape(ROWS, E)
    in_maps = []
    for i in range(N_CORES):
        xs_i = np.ascontiguousarray(
            X[i * RPC:(i + 1) * RPC, :].T.astype(ml_dtypes.bfloat16)
        )
        in_maps.append({"xs": xs_i, "wc": wc, "bw": bw_blk})
    return in_maps


def kernel(Q, K, V, Wq, bq, Wk, bk, Wv, bv, Wo, bo, **_unused):
    global LAST_RESULTS
    n, L, e = np.asarray(V).shape
    lq = float(np.asarray(Q).shape[1])
    in_maps = _prep_in_maps(V, Wv, bv, Wo, bo, lq)
    nc = _get_nc()
    LAST_RESULTS = run_bass_kernel_spmd(nc, in_maps, list(range(N_CORES)))
    out = np.concatenate(
        [
            LAST_RESULTS.results[i]["outT"].astype(np.float32).T
            for i in range(N_CORES)
        ],
        axis=0,
    )
    return np.ascontiguousarray(out).reshape(n, L, E)


# revision 6
# speedup vs baseline: 1.2126x; 1.0627x over previous
"""MultiHeadAttention kernel for 8x TRN2 NeuronCores.

The reference module's einsum reduces the attention tensor over BOTH the
query and key axes (attn_mass = sum_{q,k} softmax(logits)_k), and softmax
rows sum to 1, so attn_mass == Lq exactly for every (batch, head). The
whole computation therefore collapses to

    out = (Lq * (V_heads @ Wv^T + bv)).reshape(N, L, E) @ Wo^T + bo

which is a single dense GEMM after folding the (block-diagonal) per-head
V-projection into the output projection:

    out = V_flat @ W_eff + b_eff
    W_eff[h*hd+a, n] = Lq * sum_b Wv[b, a] * Wo[n, h*hd+b]      (1024 x 1024)
    b_eff[n]         = Lq * sum_{h,b} Wo[n, h*hd+b] * bv[b] + bo[n]

The device kernel is the GEMM, row-sharded across 8 cores (512 rows per
core), computed in TRANSPOSED orientation: out^T[n, m] = sum_k W[k, n]
X[m, k].  PSUM bank j holds output columns j*128..(j+1)*128 on partitions
x all 512 rows on the free dim, accumulating lhsT = W-block j against
rhs = X^T k-slabs.

v2 (this file): everything rides bf16 (inputs, weights, output — PSUM
still accumulates fp32; 2e-2 tolerance leaves ~5x margin), halving HBM
traffic, and the schedule is rebuilt around the two real bottlenecks the
fp32 trace exposed:

  * HAM clock ramp: the PE runs at ~1.2 GHz until it has been
    continuously busy ~4us, and a mid-stream DMA stall re-cools it
    (the fp32 run paid ~10us at half clock after stalling).  So: a
    bf16 junk-matmul burst starts the ramp right after the preamble
    and is sized so the first real matmul's inputs have landed by the
    time it drains — the PE never idles once started.
  * DMA supply: inputs stream over THREE queues (sync HWDGE, scalar
    HWDGE, gpsimd SWDGE), each tile ordered by its consumption
    deadline.  Banks 0 and 1 are interleaved (k-offset 2) so the
    X-slab consumption rate during the arrival phase is halved.
  * Output is bf16 too (host upcasts): banks evict through the vector
    engine (bias add fused, fp32->bf16) and drain on the sync queue,
    with the last bank split into quarters to shave the tail.
"""

import numpy as np
import ml_dtypes

import concourse.bass as bass
import concourse.bacc as bacc
import concourse.mybir as mybir
from concourse.tile import TileContext
from concourse.bass_utils import run_bass_kernel_spmd

N_CORES = 8
E = 1024            # embed dim == d_model
H, HD = 16, 64      # heads, head dim
ROWS = 4096         # N * L = 2 * 2048
RPC = ROWS // N_CORES   # rows per core = 512
P = 128             # SBUF partitions
KT = E // P         # 8 contraction slabs
JT = E // P         # 8 output-column banks

# Junk-matmul warm-up burst: keeps the PE busy (HAM ramp) from preamble
# exit until the first real operands land (~3.4us at the mid p-state).
N_JUNK_512 = 8
N_JUNK_128 = 2

# MM order matched to what three ~110 B/ns FIFO queues can deliver:
# banks 0..2 interleave (a new X slab or W half-chunk is consumed only
# every ~2-3 matmuls while the input stream lands), banks 3..7 run
# bank-major on fully resident X.
MM_ORDER = [
    (0, 0), (0, 1), (1, 0), (0, 2), (1, 1), (1, 2), (0, 3), (1, 3),
    (2, 0), (2, 1), (2, 2), (2, 3), (0, 4), (0, 5), (0, 6), (0, 7),
    (1, 4), (1, 5), (1, 6), (1, 7), (2, 4), (2, 5), (2, 6), (2, 7),
] + [(j, k) for j in range(3, JT) for k in range(KT)]

_NC_CACHE = {}
LAST_RESULTS = None  # BassKernelResults of the most recent device run


def _build():
    f32 = mybir.dt.float32
    bf16 = mybir.dt.bfloat16
    nc = bacc.Bacc(None, target_bir_lowering=False)
    xs = nc.declare_dram_parameter("xs", [E, RPC], bf16, isOutput=False)
    wc = nc.declare_dram_parameter("wc", [JT * P, E], bf16, isOutput=False)
    bw = nc.declare_dram_parameter("bw", [P, JT], f32, isOutput=False)
    outT = nc.declare_dram_parameter("outT", [E, RPC], bf16, isOutput=True)

    with TileContext(nc) as tc:
        with (
            tc.tile_pool(name="xp", bufs=1) as xp,
            tc.tile_pool(name="wp", bufs=1) as wp,
            tc.tile_pool(name="bp", bufs=1) as bp,
            tc.tile_pool(name="pp", bufs=1, space="PSUM") as pp,
            tc.tile_pool(name="op", bufs=1) as op,
        ):
            # Junk tile for the warm-up burst: memset needs no DMA and runs
            # first on gpsimd, so the PE can start right after the preamble
            # (a vector-side memset was measured to delay the burst ~1us).
            wm = bp.tile([P, RPC], bf16, name="wm", tag="wm")
            nc.gpsimd.memset(wm[:], 1.0)
            bias = bp.tile([P, JT], f32, name="bias", tag="bias")

            xts = [
                xp.tile([P, RPC], bf16, name=f"x{k}", tag=f"x{k}")
                for k in range(KT)
            ]

            # W chunk tiles; wmap[(j, k)] = (tile, col offset).
            wmap = {}

            def wchunk(j, k0, k1, engine):
                t = wp.tile([P, (k1 - k0) * P], bf16, name=f"w{j}_{k0}{k1}",
                            tag=f"w{j}_{k0}{k1}")
                engine.dma_start(
                    out=t[:], in_=wc[j * P:(j + 1) * P, k0 * P:k1 * P]
                )
                for k in range(k0, k1):
                    wmap[(j, k)] = (t, (k - k0) * P)

            # --- DMA schedule ------------------------------------------
            # Ordered per queue by consumption deadline against measured
            # constants: data starts flowing ~1.6-2.4us after issue, each
            # queue sustains only ~110 B/ns under full 8-core HBM
            # contention, and each dma_start costs ~0.65us of issue time
            # on its engine.
            # gpsimd SWDGE: the six W0/W1/W2 half-chunks (128KB each, one
            # landing every ~1.2us right as the interleaved banks 0..2
            # need them), then W7.
            wchunk(0, 0, 4, nc.gpsimd)
            wchunk(1, 0, 4, nc.gpsimd)
            wchunk(2, 0, 4, nc.gpsimd)
            wchunk(0, 4, 8, nc.gpsimd)
            wchunk(1, 4, 8, nc.gpsimd)
            wchunk(2, 4, 8, nc.gpsimd)
            wchunk(7, 0, 8, nc.gpsimd)
            # sync HWDGE: x0 first (gates the first real MM), then its X
            # share and the W3/W5 blocks.  Output DMAs follow.
            nc.sync.dma_start(out=xts[0][:], in_=xs[0:P, :])
            nc.sync.dma_start(out=xts[1][:], in_=xs[P:2 * P, :])
            nc.sync.dma_start(out=xts[4][:], in_=xs[4 * P:5 * P, :])
            nc.sync.dma_start(out=xts[6][:], in_=xs[6 * P:7 * P, :])
            wchunk(3, 0, 8, nc.sync)
            wchunk(5, 0, 8, nc.sync)
            # scalar HWDGE: bias (tiny, unblocks evictions), its X share,
            # then W4/W6.
            nc.scalar.dma_start(out=bias[:], in_=bw[:, :])
            nc.scalar.dma_start(out=xts[2][:], in_=xs[2 * P:3 * P, :])
            nc.scalar.dma_start(out=xts[3][:], in_=xs[3 * P:4 * P, :])
            nc.scalar.dma_start(out=xts[5][:], in_=xs[5 * P:6 * P, :])
            nc.scalar.dma_start(out=xts[7][:], in_=xs[7 * P:8 * P, :])
            wchunk(4, 0, 8, nc.scalar)
            wchunk(6, 0, 8, nc.scalar)

            ps = [
                pp.tile([P, RPC], f32, name=f"ps{j}", tag=f"ps{j}")
                for j in range(JT)
            ]

            # Warm-up burst: nonzero bf16 junk matmuls, no DMA deps.
            for i in range(N_JUNK_512):
                nc.tensor.matmul(
                    ps[i % JT], wm[:, 0:P], wm[:, :], start=True, stop=True
                )
            for i in range(N_JUNK_128):
                nc.tensor.matmul(
                    ps[(N_JUNK_512 + i) % JT][:, 0:P],
                    wm[:, 0:P], wm[:, 0:P], start=True, stop=True,
                )

            for j, k in MM_ORDER:
                t, off = wmap[(j, k)]
                nc.tensor.matmul(
                    ps[j],
                    t[:, off:off + P],
                    xts[k][:, :],
                    start=(k == 0),
                    stop=(k == KT - 1),
                )

            # Evictions: fused bias add fp32->bf16, out DMAs alternating
            # between the sync and scalar queues (both drained of inputs
            # by then).  Banks 6 and 7 evict in halves; bank 7's halves
            # run on vector and gpsimd IN PARALLEL with out DMAs on both
            # queues at once, so the tail after the last matmul is one
            # half-bank deep only.
            hh = RPC // 2
            out_q = [nc.sync, nc.scalar]
            for j in range(JT - 2):
                o = op.tile([P, RPC], bf16, name=f"o{j}", tag=f"o{j}")
                nc.vector.tensor_scalar_add(o[:], ps[j], bias[:, j:j + 1])
                out_q[j % 2].dma_start(
                    out=outT[j * P:(j + 1) * P, :], in_=o[:]
                )
            o6 = op.tile([P, RPC], bf16, name="o6", tag="o6")
            for c in range(2):
                nc.vector.tensor_scalar_add(
                    o6[:, c * hh:(c + 1) * hh],
                    ps[6][:, c * hh:(c + 1) * hh],
                    bias[:, 6:7],
                )
                out_q[c].dma_start(
                    out=outT[6 * P:7 * P, c * hh:(c + 1) * hh],
                    in_=o6[:, c * hh:(c + 1) * hh],
                )
            # Bank 7: the two halves evict on DIFFERENT engines (vector /
            # scalar-activation) and drain on different queues, fully in
            # parallel, so the post-last-matmul tail is one half deep.
            o7 = op.tile([P, RPC], bf16, name="o7", tag="o7")
            nc.vector.tensor_scalar_add(o7[:, 0:hh], ps[7][:, 0:hh],
                                        bias[:, 7:8])
            nc.sync.dma_start(out=outT[7 * P:8 * P, 0:hh], in_=o7[:, 0:hh])
            nc.scalar.add(o7[:, hh:RPC], ps[7][:, hh:RPC], bias[:, 7:8])
            nc.scalar.dma_start(out=outT[7 * P:8 * P, hh:RPC],
                                in_=o7[:, hh:RPC])
    nc.compile()
    return nc


def _get_nc():
    if "bf16" not in _NC_CACHE:
        _NC_CACHE["bf16"] = _build()
    return _NC_CACHE["bf16"]


def _prep_in_maps(V, Wv, bv, Wo, bo, lq):
    V = np.ascontiguousarray(np.asarray(V, dtype=np.float32))
    Wv64 = np.asarray(Wv, np.float64)
    Wo64 = np.asarray(Wo, np.float64)
    bv64 = np.asarray(bv, np.float64)
    bo64 = np.asarray(bo, np.float64)

    # Fold per-head V-projection + output projection + attention mass (== Lq).
    Wo_r = Wo64.reshape(E, H, HD)                       # [n, h, b]
    W_eff = lq * np.einsum("ba,nhb->han", Wv64, Wo_r, optimize=True)
    W_eff = W_eff.reshape(E, E).astype(np.float32)      # [k, n]
    b_eff = (lq * np.einsum("nhb,b->n", Wo_r, bv64) + bo64).astype(np.float32)

    # wc[j*P + p, k*P + c] = W_eff[k*P + p, j*P + c]  (lhsT blocks, natural)
    wc = np.ascontiguousarray(
        W_eff.reshape(KT, P, JT, P).transpose(2, 1, 0, 3).reshape(JT * P, E)
    ).astype(ml_dtypes.bfloat16)
    bw_blk = np.ascontiguousarray(b_eff.reshape(JT, P).T)   # [p, j] fp32

    X = V.reshape(ROWS, E)
    in_maps = []
    for i in range(N_CORES):
        xs_i = np.ascontiguousarray(
            X[i * RPC:(i + 1) * RPC, :].T.astype(ml_dtypes.bfloat16)
        )
        in_maps.append({"xs": xs_i, "wc": wc, "bw": bw_blk})
    return in_maps


def kernel(Q, K, V, Wq, bq, Wk, bk, Wv, bv, Wo, bo, **_unused):
    global LAST_RESULTS
    n, L, e = np.asarray(V).shape
    lq = float(np.asarray(Q).shape[1])
    in_maps = _prep_in_maps(V, Wv, bv, Wo, bo, lq)
    nc = _get_nc()
    LAST_RESULTS = run_bass_kernel_spmd(nc, in_maps, list(range(N_CORES)))
    out = np.concatenate(
        [
            LAST_RESULTS.results[i]["outT"].astype(np.float32).T
            for i in range(N_CORES)
        ],
        axis=0,
    )
    return np.ascontiguousarray(out).reshape(n, L, E)
